# revision 13
# baseline (speedup 1.0000x reference)
"""Distributed Trainium2 Bass kernel for nn_ActorGCN (GNN message passing).

8 NeuronCores, SPMD, node-octile sharding:
  - Edge MLP over a src-sorted bucket-padded edge stream (features on
    partitions); segment sums via strided VectorE reduces.
  - GCN aggregations use linearity (A@(hW) == (A@h)@W): gathers move only
    30/64-wide node features via GPSIMD ap_gather against octile-split
    tables (one node octile per Q7 16-partition group), per-group partial
    sums combined by a tiny selector matmul.
  - AllGather distributes bf16 node tables between phases; final AllReduce
    combines per-core column sums.

Host does only index/layout preprocessing; all float math is on device.
"""
import sys
import numpy as np

sys.path.insert(0, "/opt/trn_rl_repo")

N = 50000
E = 1600000
NCORE = 8
NSH = N // NCORE            # 6250
NSHP = NSH + 6              # 6256 (mult of 16; rows NSH.. are zero)
F0 = 32
H = 128
F2 = 64

B0 = [4, 8, 12, 16, 20, 24, 28, 32, 36, 40, 44, 48, 56, 64, 80, 96, 128]
B1 = [2, 4, 6, 8, 10, 12, 16, 20, 24, 32, 48, 64]

SC0 = 4096
SC1 = 1024

PERM32 = np.concatenate([np.arange(0, F0, 2), np.arange(1, F0, 2)])
PERM64 = np.concatenate([np.arange(j, F2, 4) for j in range(4)])


def _bucket_vec(deg, B):
    K = np.full(deg.shape, B[-1], np.int64)
    for b in reversed(B):
        K[deg <= b] = b
    assert (deg <= K).all()
    return K


def _round8(v):
    return (int(v) + 7) // 8 * 8


def _wrap16(vals, parts, base_part=0):
    n = len(vals)
    assert n % 16 == 0
    w = n // 16
    out = np.zeros((parts, w), np.int16)
    out[base_part:base_part + 16, :] = np.asarray(vals, np.int16).reshape(w, 16).T
    return out


def _chunks(layout, cap):
    out = []
    node_off = 0
    slot_off = 0
    for b, cnt in layout:
        left = cnt
        mstep = max(8, (cap // b) // 8 * 8)
        while left > 0:
            m = min(left, mstep)
            out.append((b, m, node_off, slot_off))
            node_off += m
            slot_off += b * m
            left -= m
    return out


def host_prep(inputs):
    import ml_dtypes
    bft = ml_dtypes.bfloat16
    x = np.asarray(inputs["x"], np.float32)
    ei = np.asarray(inputs["edge_index"])
    ea = np.asarray(inputs["edge_attr"], np.float32)
    src = ei[0].astype(np.int64)
    dst = ei[1].astype(np.int64)
    W = {k: np.asarray(inputs[k], np.float32) for k in
         ["W_e1", "b_e1", "W_e2", "b_e2", "W_g1", "b_g1", "W_g2", "b_g2",
          "W_a", "b_a", "W_o", "b_o"]}

    We2p = np.zeros((H, F0), np.float32); We2p[:, :30] = W["W_e2"]
    We2p = We2p[:, PERM32]
    be2p = np.zeros((F0,), np.float32); be2p[:30] = W["b_e2"]
    be2p = be2p[PERM32]
    Wg1p = np.zeros((F0, H), np.float32); Wg1p[:30] = W["W_g1"]
    Wg1p = Wg1p[PERM32]
    Wg2p = W["W_g2"][:, PERM64]
    bg2p = W["b_g2"][PERM64]
    Wap = W["W_a"][PERM64]
    bg2_j = np.stack([bg2p[16 * j:16 * (j + 1)] for j in range(4)], 1)  # [16,4]

    deg_dst = np.bincount(dst, minlength=N).astype(np.float32)

    # ---------- MP0 grids ----------
    per_core0 = []
    for c in range(NCORE):
        lo = c * NSH
        eidx = np.nonzero((src >= lo) & (src < lo + NSH))[0]
        s_loc = src[eidx] - lo
        order = np.argsort(s_loc, kind="stable")
        eidx = eidx[order]
        deg = np.bincount(s_loc, minlength=NSH)
        K = _bucket_vec(np.maximum(deg, 1), B0)
        per_core0.append((eidx, deg, K))
    bc0 = {b: _round8(max(int((K == b).sum()) for _, _, K in per_core0))
           for b in B0}
    layout0 = [(b, bc0[b]) for b in B0 if bc0[b] > 0]
    NG0 = sum(cnt for _, cnt in layout0)
    slots0 = sum(b * cnt for b, cnt in layout0)
    plan0 = _chunks(layout0, SC0)

    # ---------- MP1/MP2 grids ----------
    per_cg = {}
    for c in range(NCORE):
        lo = c * NSH
        eidx = np.nonzero((dst >= lo) & (dst < lo + NSH))[0]
        g_of = src[eidx] // NSH
        for g in range(NCORE):
            e2 = eidx[g_of == g]
            d_loc = dst[e2] - lo
            order = np.argsort(d_loc, kind="stable")
            e2 = e2[order]
            sdeg = np.bincount(d_loc, minlength=NSH)
            K = _bucket_vec(np.maximum(sdeg, 1), B1)
            per_cg[(c, g)] = (e2, sdeg, K)
    bc1 = {b: _round8(max(int((K == b).sum()) for _, _, K in per_cg.values()))
           for b in B1}
    layout1 = [(b, bc1[b]) for b in B1 if bc1[b] > 0]
    NG1 = sum(cnt for _, cnt in layout1)
    NG1T = _round8(NG1)
    slots1 = sum(b * cnt for b, cnt in layout1)
    plan1 = _chunks(layout1, SC1)

    in_maps = []
    for c in range(NCORE):
        m = {}
        lo = c * NSH
        # ---- MP0 stream ----
        eidx, deg, K0 = per_core0[c]
        grid_nodes = np.full(NG0, -1, np.int64)
        pos = 0
        for b, cnt in layout0:
            nodes = np.nonzero(K0 == b)[0]
            grid_nodes[pos:pos + len(nodes)] = nodes
            pos += cnt
        run_b = np.concatenate([np.full(cnt, b) for b, cnt in layout0])
        run_start = np.concatenate([[0], np.cumsum(run_b)[:-1]])
        gp_of_node = np.zeros(NSH, np.int64)
        valid = grid_nodes >= 0
        gp_of_node[grid_nodes[valid]] = np.nonzero(valid)[0]
        s_loc = src[eidx] - lo
        first = np.concatenate([[0], np.cumsum(deg)[:-1]])
        within = np.arange(len(eidx)) - first[s_loc]
        slot = run_start[gp_of_node[s_loc]] + within
        ea_pad = np.zeros((slots0, 16), np.float32)
        ea_pad[slot] = ea[eidx]
        m["ea"] = np.ascontiguousarray(ea_pad.T).astype(bft)
        permv = np.zeros(NSHP, np.int64)
        permv[:NSH] = gp_of_node
        m["perm0"] = np.concatenate([_wrap16(permv, 16)] * 2, 0)
        m["k32"] = np.broadcast_to(K0.astype(np.float32), (F0, NSH)).copy()
        m["cnt32"] = np.broadcast_to(deg.astype(np.float32), (F0, NSH)).copy()
        m["degd16"] = np.broadcast_to(deg_dst[lo:lo + NSH], (16, NSH)).copy()
        xp = np.zeros((F0, NSH), np.float32)
        xp[:30] = x[lo:lo + NSH].T
        m["x32"] = np.ascontiguousarray(xp[PERM32]).astype(bft)
        # ---- MP1/2 idx + perm ----
        idx1 = np.zeros((128, slots1 // 16), np.int16)
        perm1 = np.zeros((128, NSHP // 16), np.int16)
        for g in range(NCORE):
            e2, sdeg, K1 = per_cg[(c, g)]
            gn = np.full(NG1, -1, np.int64)
            pos = 0
            for b, cnt in layout1:
                nodes = np.nonzero(K1 == b)[0]
                gn[pos:pos + len(nodes)] = nodes
                pos += cnt
            run_b1 = np.concatenate([np.full(cnt, b) for b, cnt in layout1])
            rs1 = np.concatenate([[0], np.cumsum(run_b1)[:-1]])
            gpn = np.zeros(NSH, np.int64)
            vv = gn >= 0
            gpn[gn[vv]] = np.nonzero(vv)[0]
            d_loc = dst[e2] - lo
            first = np.concatenate([[0], np.cumsum(sdeg)[:-1]])
            within = np.arange(len(e2)) - first[d_loc]
            slotv = np.full(slots1, NSH, np.int64)
            slotv[rs1[gpn[d_loc]] + within] = src[e2] - g * NSH
            idx1 += _wrap16(slotv, 128, base_part=16 * g)
            pv = np.zeros(NSHP, np.int64)
            pv[:NSH] = gpn
            perm1 += _wrap16(pv, 128, base_part=16 * g)
        m["idx1"] = idx1
        m["perm1"] = perm1
        # ---- weights ----
        m["We1"] = W["W_e1"].astype(bft)
        m["We2"] = We2p.astype(bft)
        m["be1"] = W["b_e1"].reshape(H, 1).copy()
        m["be2"] = be2p.reshape(F0, 1).copy()
        m["Wg1a"] = Wg1p[0:16].astype(bft)
        m["Wg1b"] = Wg1p[16:32].astype(bft)
        m["bg1"] = W["b_g1"].reshape(H, 1).copy()
        m["Wg2"] = Wg2p.astype(bft)
        m["bg2j"] = np.ascontiguousarray(bg2_j)
        for j in range(4):
            m[f"Wa{j}"] = np.ascontiguousarray(Wap[16 * j:16 * (j + 1)]).astype(bft)
        m["ba"] = W["b_a"].reshape(F2, 1).copy()
        m["Wo"] = W["W_o"].copy()
        m["bo"] = W["b_o"].reshape(16, 1).copy()
        sel = np.zeros((128, 16), np.float32)
        for g in range(NCORE):
            sel[16 * g + np.arange(16), np.arange(16)] = 1.0
        m["sel"] = sel.astype(bft)
        in_maps.append(m)

    plan = dict(layout0=layout0, NG0=NG0, slots0=slots0, plan0=plan0,
                layout1=layout1, NG1=NG1, NG1T=NG1T, slots1=slots1,
                plan1=plan1)
    return in_maps, plan


def build(plan):
    from concourse import bass, bacc, tile
    from concourse.bass import mybir
    dt = mybir.dt
    AF = mybir.ActivationFunctionType
    ALU = mybir.AluOpType
    X = mybir.AxisListType.X

    NG0, slots0, plan0 = plan["NG0"], plan["slots0"], plan["plan0"]
    NG1T, slots1, plan1 = plan["NG1T"], plan["slots1"], plan["plan1"]
    NG0P = _round8(NG0)

    nc = bacc.Bacc("TRN2", target_bir_lowering=False, debug=False,
                   num_devices=NCORE)

    def par(name, shape, dtype=dt.float32):
        return nc.declare_dram_parameter(name, list(shape), dtype, False)

    ea = par("ea", [16, slots0], dt.bfloat16)
    perm0 = par("perm0", [32, NSHP // 16], dt.int16)
    k32 = par("k32", [F0, NSH]); cnt32 = par("cnt32", [F0, NSH])
    degd16 = par("degd16", [16, NSH])
    x32 = par("x32", [F0, NSH], dt.bfloat16)
    idx1 = par("idx1", [128, slots1 // 16], dt.int16)
    perm1 = par("perm1", [128, NSHP // 16], dt.int16)
    We1 = par("We1", [16, H], dt.bfloat16)
    We2 = par("We2", [H, F0], dt.bfloat16)
    be1 = par("be1", [H, 1]); be2 = par("be2", [F0, 1])
    Wg1a = par("Wg1a", [16, H], dt.bfloat16)
    Wg1b = par("Wg1b", [16, H], dt.bfloat16)
    bg1 = par("bg1", [H, 1])
    Wg2 = par("Wg2", [H, F2], dt.bfloat16)
    bg2j = par("bg2j", [16, 4])
    Wa = [par(f"Wa{j}", [16, F2], dt.bfloat16) for j in range(4)]
    ba = par("ba", [F2, 1]); Wo = par("Wo", [F2, 16]); bo = par("bo", [16, 1])
    sel = par("sel", [128, 16], dt.bfloat16)
    out = nc.declare_dram_parameter("out", [1, 16], dt.float32, True)

    # collective + staging DRAM buffers
    pin0 = nc.dram_tensor("pin0", [16, NSHP * 2], dt.bfloat16)
    pout0 = nc.dram_tensor("pout0", [NCORE, 16, NSHP * 2], dt.bfloat16,
                           addr_space="Shared")
    pin2 = nc.dram_tensor("pin2", [2, 16, NSHP * 2], dt.bfloat16)
    pout2 = nc.dram_tensor("pout2", [NCORE, 2, 16, NSHP * 2], dt.bfloat16,
                           addr_space="Shared")
    arin = nc.dram_tensor("arin", [F2, 1], dt.float32)
    arout = nc.dram_tensor("arout", [F2, 1], dt.float32, addr_space="Shared")
    h2d = nc.dram_tensor("h2d", [16, NSH * 4], dt.bfloat16)
    ddram = nc.dram_tensor("ddram", [16, NSH], dt.float32)

    RG = [list(range(NCORE))]

    with tile.TileContext(nc) as tc:
        with (
            tc.tile_pool(name="const", bufs=1) as cpool,
            tc.tile_pool(name="psA", bufs=1, space="PSUM") as psA,
        ):
            def load(h, pool=None, tag=None):
                t = (pool or cpool).tile(list(h.shape), h.dtype, tag=tag or h.name)
                nc.sync.dma_start(out=t[:], in_=h[:])
                return t

            We1_s = load(We1); We2_s = load(We2)
            be1_s = load(be1); be2_s = load(be2)
            idx1_s = load(idx1); perm1_s = load(perm1); perm0_s = load(perm0)
            sel_s = load(sel)
            bg2j_s = load(bg2j); ba_s = load(ba)
            Wo_s = load(Wo); bo_s = load(bo)

            dinv16 = cpool.tile([16, NSH], dt.float32)
            with tc.tile_pool(name="dgtmp", bufs=1) as dg:
                degd16_s = load(degd16, dg)
                nc.vector.tensor_scalar_add(out=dinv16[:], in0=degd16_s[:],
                                            scalar1=1.0)
            nc.vector.reciprocal(out=dinv16[:], in_=dinv16[:])
            nc.scalar.sqrt(out=dinv16[:], in_=dinv16[:])
            nc.sync.dma_start(out=ddram[:], in_=dinv16[:])

            # v_pad = relu(We2p.T @ relu(be1) + be2)
            rb = cpool.tile([H, 1], dt.bfloat16)
            nc.scalar.activation(out=rb[:], in_=be1_s[:], func=AF.Relu)
            vp_ps = psA.tile([F0, 1], dt.float32, tag="vp")
            nc.tensor.matmul(out=vp_ps[:], lhsT=We2_s[:], rhs=rb[:],
                             start=True, stop=True)
            v_pad = cpool.tile([F0, 1], dt.float32)
            nc.scalar.activation(out=v_pad[:], in_=vp_ps[:], func=AF.Relu,
                                 bias=be2_s[:])

            # ---------------- Phase 0: edge MLP + segment reduce ----------
            with tc.tile_pool(name="p0c", bufs=1) as p0c:
              sum0c = p0c.tile([F0, NSHP], dt.float32)
              with tc.tile_pool(name="p0", bufs=1) as p0:
                sum0 = p0.tile([F0, NG0P], dt.float32)
                with (
                    tc.tile_pool(name="mlp", bufs=2) as mp,
                    tc.tile_pool(name="ps0", bufs=3, space="PSUM") as ps0,
                ):
                    for (b, m, node_off, slot_off) in plan0:
                        n = b * m
                        eat = mp.tile([16, SC0], dt.bfloat16, tag="ea")
                        nc.sync.dma_start(out=eat[:, :n],
                                          in_=ea[:, slot_off:slot_off + n])
                        ef1 = mp.tile([H, SC0], dt.bfloat16, tag="ef1")
                        for j in range(0, n, 512):
                            w = min(512, n - j)
                            pt = ps0.tile([H, 512], dt.float32, tag="ps1")
                            nc.tensor.matmul(out=pt[:, :w], lhsT=We1_s[:],
                                             rhs=eat[:, j:j + w],
                                             start=True, stop=True)
                            nc.scalar.activation(out=ef1[:, j:j + w],
                                                 in_=pt[:, :w], func=AF.Relu,
                                                 bias=be1_s[:])
                        ef2 = mp.tile([F0, SC0], dt.bfloat16, tag="ef2")
                        for j in range(0, n, 512):
                            w = min(512, n - j)
                            pt2 = ps0.tile([F0, 512], dt.float32, tag="ps2")
                            nc.tensor.matmul(out=pt2[:, :w], lhsT=We2_s[:],
                                             rhs=ef1[:, j:j + w],
                                             start=True, stop=True)
                            nc.vector.tensor_scalar(
                                out=ef2[:, j:j + w], in0=pt2[:, :w],
                                scalar1=be2_s[:], scalar2=0.0,
                                op0=ALU.add, op1=ALU.max)
                        nc.vector.tensor_reduce(
                            out=sum0[:, node_off:node_off + m],
                            in_=ef2[:, :n].rearrange("p (m b) -> p m b", m=m),
                            axis=X, op=ALU.add)

                # permute grid -> canonical
                nc.gpsimd.ap_gather(out_ap=sum0c[:], in_ap=sum0[:],
                                    idxs_ap=perm0_s[:], channels=F0,
                                    num_elems=NG0P, d=1, num_idxs=NSHP)
              with tc.tile_pool(name="p0b", bufs=1) as p0:
                k32_s = load(k32, p0); cnt32_s = load(cnt32, p0)
                x32_s = load(x32, p0)
                h32 = p0.tile([F0, NSH], dt.float32)
                tmp = p0.tile([F0, NSH], dt.float32)
                dinv32 = p0.tile([F0, NSH], dt.float32)
                for hh in range(2):
                    nc.sync.dma_start(out=dinv32[16 * hh:16 * hh + 16, :],
                                      in_=ddram[:])
                nc.vector.tensor_tensor(out=tmp[:], in0=k32_s[:],
                                        in1=cnt32_s[:], op=ALU.subtract)
                nc.vector.tensor_tensor(
                    out=tmp[:], in0=tmp[:],
                    in1=v_pad[:].broadcast_to([F0, NSH]), op=ALU.mult)
                nc.vector.tensor_tensor(out=h32[:], in0=sum0c[:, :NSH],
                                        in1=tmp[:], op=ALU.subtract)
                nc.vector.tensor_scalar_max(out=tmp[:], in0=cnt32_s[:],
                                            scalar1=1.0)
                nc.vector.reciprocal(out=tmp[:], in_=tmp[:])
                nc.vector.tensor_tensor(out=h32[:], in0=h32[:], in1=tmp[:],
                                        op=ALU.mult)
                nc.vector.tensor_tensor(out=h32[:], in0=h32[:], in1=x32_s[:],
                                        op=ALU.add)
                nc.vector.tensor_tensor(out=h32[:], in0=h32[:], in1=dinv32[:],
                                        op=ALU.mult)
                pbf = p0.tile([F0, NSH], dt.bfloat16)
                nc.vector.tensor_copy(out=pbf[:], in_=h32[:])

                z16 = cpool.tile([16, 16], dt.bfloat16)
                nc.vector.memset(z16[:], 0.0)
                with nc.allow_non_contiguous_dma("pair fold"):
                    nc.sync.dma_start(
                        out=pin0[:].rearrange("p (n d) -> p n d", d=2)[:, :NSH, 0],
                        in_=pbf[0:16, :])
                    nc.sync.dma_start(
                        out=pin0[:].rearrange("p (n d) -> p n d", d=2)[:, :NSH, 1],
                        in_=pbf[16:32, :])
                nc.sync.dma_start(out=pin0[:, 2 * NSH:],
                                  in_=z16[:, :2 * (NSHP - NSH)])
            nc.gpsimd.collective_compute(
                "AllGather", ALU.bypass, replica_groups=RG,
                ins=[pin0[:].opt()], outs=[pout0[:].opt()])

            # ---------------- MP1 ----------------
            def mp_gather(table_src_blocks, qcan_tag):
                """Gather+reduce+permute; returns canonical q [128, NSHP, 2]
                (bf16) tile."""
                qgrid = mpool.tile([128, NG1T, 2], dt.bfloat16, tag="qgrid")
                with (
                    tc.tile_pool(name="gt", bufs=1) as gt,
                    tc.tile_pool(name="gch", bufs=2) as gchp,
                ):
                    table = gt.tile([128, NSHP, 2], dt.bfloat16, tag="table")
                    for g in range(NCORE):
                        nc.sync.dma_start(
                            out=table[16 * g:16 * (g + 1), :, :],
                            in_=table_src_blocks(g))
                    with nc.allow_low_precision("bf16 grid"):
                        for (b, m, node_off, slot_off) in plan1:
                            n = b * m
                            gch = gchp.tile([128, SC1, 2], dt.bfloat16, tag="gch")
                            nc.gpsimd.ap_gather(
                                out_ap=gch[:, :n, :], in_ap=table[:],
                                idxs_ap=idx1_s[:, slot_off // 16:
                                               (slot_off + n) // 16],
                                channels=128, num_elems=NSHP, d=2, num_idxs=n)
                            for j in range(2):
                                nc.vector.tensor_reduce(
                                    out=qgrid[:, node_off:node_off + m, j],
                                    in_=gch[:, :n, j].rearrange(
                                        "p (m b) -> p m b", m=m),
                                    axis=X, op=ALU.add)
                qcan = mpool.tile([128, NSHP, 2], dt.bfloat16, tag=qcan_tag)
                with nc.allow_low_precision("bf16 permute"):
                    nc.gpsimd.ap_gather(out_ap=qcan[:], in_ap=qgrid[:],
                                        idxs_ap=perm1_s[:], channels=128,
                                        num_elems=NG1T, d=2, num_idxs=NSHP)
                return qcan

            with (
                tc.tile_pool(name="mp", bufs=1) as mpool,
                tc.tile_pool(name="ps1p", bufs=2, space="PSUM") as ps1p,
                tc.tile_pool(name="sm", bufs=2) as sm,
            ):
                qcan = mp_gather(
                    lambda g: pout0[g, :, :].rearrange("p (n d) -> p n d", d=2),
                    "qcan")
                ppair = mpool.tile([16, NSH, 2], dt.bfloat16, tag="ppair")
                nc.sync.dma_start(
                    out=ppair[:],
                    in_=pin0[:].rearrange("p (n d) -> p n d", d=2)[:, :NSH, :])
                m1 = ppair
                for o in range(0, NSH, 256):
                    w = min(256, NSH - o)
                    qp = ps1p.tile([16, 512], dt.float32, tag="gs")
                    nc.tensor.matmul(
                        out=qp[:, :2 * w], lhsT=sel_s[:],
                        rhs=qcan[:, o:o + w, :].rearrange("p n d -> p (n d)"),
                        start=True, stop=True)
                    t = sm.tile([16, 512], dt.float32, tag="t")
                    nc.vector.tensor_tensor(
                        out=t[:, :2 * w], in0=qp[:, :2 * w],
                        in1=ppair[:, o:o + w, :].rearrange("p n d -> p (n d)"),
                        op=ALU.add)
                    t3 = t[:, :2 * w].rearrange("p (n d) -> p n d", d=2)
                    nc.vector.tensor_tensor(
                        out=m1[:, o:o + w, :], in0=t3,
                        in1=dinv16[:, o:o + w].unsqueeze(-1)
                        .broadcast_to([16, w, 2]),
                        op=ALU.mult)
                # h1 = relu(Wg1a.T@m1[...,0] + Wg1b.T@m1[...,1] + bg1)
                Wg1a_s = load(Wg1a, mpool); Wg1b_s = load(Wg1b, mpool)
                bg1_s = load(bg1, mpool); Wg2_s = load(Wg2, mpool)
                h1 = mpool.tile([H, NSH], dt.bfloat16, tag="h1")
                for o in range(0, NSH, 512):
                    w = min(512, NSH - o)
                    hp = ps1p.tile([H, 512], dt.float32, tag="h1p")
                    nc.tensor.matmul(out=hp[:, :w], lhsT=Wg1a_s[:],
                                     rhs=m1[:, o:o + w, 0],
                                     start=True, stop=False)
                    nc.tensor.matmul(out=hp[:, :w], lhsT=Wg1b_s[:],
                                     rhs=m1[:, o:o + w, 1],
                                     start=False, stop=True)
                    nc.scalar.activation(out=h1[:, o:o + w], in_=hp[:, :w],
                                         func=AF.Relu, bias=bg1_s[:])
                # p2 = dinv * (h1.T @ Wg2p), one 16-wide block per j,
                # streamed straight into the collective input buffer
                with nc.allow_non_contiguous_dma("quad fold"):
                    for o in range(0, NSH, 512):
                        w = min(512, NSH - o)
                        for j in range(4):
                            zp = ps1p.tile([16, 512], dt.float32, tag="zp")
                            nc.tensor.matmul(out=zp[:, :w],
                                             lhsT=Wg2_s[:, 16 * j:16 * (j + 1)],
                                             rhs=h1[:, o:o + w],
                                             start=True, stop=True)
                            p2t = sm.tile([16, 512], dt.bfloat16, tag="p2t")
                            nc.vector.tensor_tensor(
                                out=p2t[:, :w], in0=zp[:, :w],
                                in1=dinv16[:, o:o + w], op=ALU.mult)
                            nc.sync.dma_start(
                                out=pin2[j // 2, :, :].rearrange(
                                    "p (n d) -> p n d", d=2)[:, o:o + w, j % 2],
                                in_=p2t[:, :w])
                    for half in range(2):
                        nc.sync.dma_start(out=pin2[half, :, 2 * NSH:],
                                          in_=z16[:, :2 * (NSHP - NSH)])
            nc.gpsimd.collective_compute(
                "AllGather", ALU.bypass, replica_groups=RG,
                ins=[pin2[:].opt()], outs=[pout2[:].opt()])

            # ---------------- MP2 ----------------
            with (
                tc.tile_pool(name="mp2", bufs=1) as mpool,
                tc.tile_pool(name="ps2p", bufs=2, space="PSUM") as ps2p,
                tc.tile_pool(name="sm2", bufs=2) as sm2,
            ):
                for half in range(2):
                    qcan2 = mp_gather(
                        lambda g, _h=half: pout2[g, _h, :, :]
                        .rearrange("p (n d) -> p n d", d=2),
                        "qcan2")
                    p2h = []
                    with nc.allow_non_contiguous_dma("p2 reload"):
                        for j in range(2):
                            p2hj = mpool.tile([16, NSH], dt.bfloat16,
                                              tag=f"p2h{j}")
                            nc.sync.dma_start(
                                out=p2hj[:],
                                in_=pin2[half, :, :].rearrange(
                                    "p (n d) -> p n d", d=2)[:, :NSH, j])
                            p2h.append(p2hj)
                    for o in range(0, NSH, 256):
                        w = min(256, NSH - o)
                        qp = ps2p.tile([16, 512], dt.float32, tag="gs2")
                        nc.tensor.matmul(
                            out=qp[:, :2 * w], lhsT=sel_s[:],
                            rhs=qcan2[:, o:o + w, :]
                            .rearrange("p n d -> p (n d)"),
                            start=True, stop=True)
                        t = sm2.tile([16, 512], dt.float32, tag="t2")
                        t3 = t[:, :2 * w].rearrange("p (n d) -> p n d", d=2)
                        qp3 = qp[:, :2 * w].rearrange("p (n d) -> p n d", d=2)
                        for j in range(2):
                            nc.vector.tensor_tensor(
                                out=t3[:, :, j], in0=qp3[:, :, j],
                                in1=p2h[j][:, o:o + w], op=ALU.add)
                        nc.vector.tensor_tensor(
                            out=t3, in0=t3,
                            in1=dinv16[:, o:o + w].unsqueeze(-1)
                            .broadcast_to([16, w, 2]), op=ALU.mult)
                        nc.vector.tensor_tensor(
                            out=t3, in0=t3,
                            in1=bg2j_s[:, 2 * half:2 * half + 2].unsqueeze(1)
                            .broadcast_to([16, w, 2]), op=ALU.add)
                        h2sb = sm2.tile([16, 512], dt.bfloat16, tag="h2sb")
                        nc.vector.tensor_scalar_max(out=h2sb[:, :2 * w],
                                                    in0=t[:, :2 * w],
                                                    scalar1=0.0)
                        with nc.allow_non_contiguous_dma("h2 slices"):
                            nc.sync.dma_start(
                                out=h2d[:].rearrange("p (n d) -> p n d", d=4)
                                [:, o:o + w, 2 * half:2 * half + 2],
                                in_=h2sb[:, :2 * w]
                                .rearrange("p (n d) -> p n d", d=2))

                # a = relu(sum_j Wa_j.T @ h2[...,j] + ba); colsum
                Wa_s = [load(w, mpool) for w in Wa]
                asum = mpool.tile([F2, 1], dt.float32, tag="asum")
                ab = mpool.tile([F2, NSH], dt.bfloat16, tag="ab")
                for o in range(0, NSH, 512):
                    w = min(512, NSH - o)
                    h2c = sm2.tile([16, 512 * 4], dt.bfloat16, tag="h2c")
                    nc.sync.dma_start(out=h2c[:, :4 * w],
                                      in_=h2d[:, 4 * o:4 * (o + w)])
                    h2c3 = h2c[:, :4 * w].rearrange("p (n d) -> p n d", d=4)
                    ap = ps2p.tile([F2, 512], dt.float32, tag="ap")
                    for j in range(4):
                        nc.tensor.matmul(out=ap[:, :w], lhsT=Wa_s[j][:],
                                         rhs=h2c3[:, :, j],
                                         start=(j == 0), stop=(j == 3))
                    nc.scalar.activation(out=ab[:, o:o + w], in_=ap[:, :w],
                                         func=AF.Relu, bias=ba_s[:])
                nc.vector.tensor_reduce(out=asum[:], in_=ab[:], axis=X,
                                        op=ALU.add)
                nc.sync.dma_start(out=arin[:], in_=asum[:])
            nc.gpsimd.collective_compute(
                "AllReduce", ALU.add, replica_groups=RG,
                ins=[arin[:].opt()], outs=[arout[:].opt()])
            with tc.tile_pool(name="fin", bufs=1) as fin:
                s = fin.tile([F2, 1], dt.float32)
                nc.sync.dma_start(out=s[:], in_=arout[:])
                nc.scalar.mul(out=s[:], in_=s[:], mul=1.0 / N)
                op = psA.tile([16, 1], dt.float32, tag="op")
                nc.tensor.matmul(out=op[:], lhsT=Wo_s[:], rhs=s[:],
                                 start=True, stop=True)
                o_sb = fin.tile([16, 1], dt.float32)
                nc.scalar.activation(out=o_sb[:], in_=op[:],
                                     func=AF.Identity, bias=bo_s[:])
                nc.sync.dma_start(out=out[:].rearrange("o p -> p o"), in_=o_sb[:])
    nc.compile()
    return nc


def kernel(trace=False, **inputs):
    from concourse.bass_utils import run_bass_kernel_spmd
    in_maps, plan = host_prep(inputs)
    nc = build(plan)
    res = run_bass_kernel_spmd(nc, in_maps, core_ids=list(range(NCORE)),
                               trace=trace)
    o = np.asarray(res.results[0]["out"], np.float32).reshape(16)
    if trace:
        return o, res
    return o
